# revision 129
# baseline (speedup 1.0000x reference)
"""Multi-head attention (16 heads, N=2048, D=1024, E=64) on 8 Trainium2 cores.

Head-parallel sharding: core m handles heads (2m, 2m+1), computes its two
heads' attention contexts and a partial o_proj (rows 128m:128m+128 of the
row-sharded o_proj); the host sums the 8 partial fp32 outputs in fp64.

Numerics (unchanged from the validated baseline): the softmax-score path
is fp32-accurate via an exact hi/lo float32r decomposition (host RNE-11
rounding == the PE's f32r operand rounding); v/ctx/o_proj tolerate plain
f32r. Scores per [m=128, q=512] tile are two f32r matmuls: a stacked
K=128 cross-term matmul [kl;kh]@[qh;ql] plus a K=65 kh_ext@qh_ext matmul
whose row 64 carries -rowmax(q) (computed by a separate hi-only max pass
in [q, m] layout, DVE-reduced); exp((S-c)/8) on ACT; ctx^T/Z accumulate
on PE via a ones-column in v_ext; 1/Z broadcast+mul normalizes.

Schedule (restructured around the cost model):
  - x is loaded ONCE as fp32 [128, N, 8] (c-interleaved, 8 DMAs of
    1MB, half the bytes of the old host-split hi/lo pair) and split
    into f32r hi/lo on device (DVE round-copy + GpSimd subtract — the
    device f32r convert equals the PE's operand rounding).
  - v projection is hi-only (1 matmul per d-chunk instead of 3): v is
    rounded to f32r downstream anyway, so the extra ~5e-4 relative
    error is far inside the 2e-2 budget.
  - no qTr/kTr intermediates: the hi parts live in known partition rows
    of qx/kx and the max pass reads them there.
  - PSUM is partitioned into dedicated pools (scores+oproj 3+1 banks,
    ctx 2, max-pass paired [128,2,512] tiles 2-4) so a lagging DVE
    reduce can never lock a bank the score matmuls need.
  - max-pass reduces are PAIRED (one XY-reduce over two 512-wide
    m-chunks) and spread so DVE never lock-steps an idle PE: pair(0,1)
    groups ride projection chunks 3-6 (4 units each), chunk 0's
    pair(2,3) group interleaves chunk 7's q matmuls, chunk 1's rides
    attention(0)'s first tiles, and mp(2)/mp(3) splits across
    attention(0)/(1)/(2) with late finish staging.
  - GpSimd (no PSUM port) takes the SBUF-only copies/subs; the q/k
    unpack runs on ACT (fp32 scratch + rounded hi) + GpSimd (lo
    residual, ext copy); DVE keeps the reduces and the split copies.
  - score ext matmuls lag TWO tiles behind their cross matmuls (the
    third score psum bank buys exp/ctx an extra ~0.8us of pipeline
    slack per tile).
  - the drain is pipelined: the last chunk runs its heads sequentially,
    the final normalize is emitted in 128-col slices, and the last
    o_proj blocks rotate through the score psum banks with per-half
    output DMAs on both HWDGE queues.
"""
import sys

sys.path.insert(0, "/opt/trn_rl_repo")

from contextlib import ExitStack

import numpy as np

import concourse.bass as bass
import concourse.mybir as mybir
import concourse.tile as tile
from concourse import bacc
from concourse.bass_utils import run_bass_kernel_spmd
from concourse.masks import make_identity

# problem shapes (hardcoded per contract)
N = 2048
D = 1024
E = 64
H = 16
N_CORES = 8
H_PER_CORE = H // N_CORES  # 2

QC = 512          # q-chunk (moving dim of S'/ctx matmuls)
NQ = N // QC      # 4
MB = 128          # m-block (partition dim of S'^T tiles)
NMB = N // MB     # 16
DCH = D // 128    # 8 d-chunks for projections
PC = 256          # projection n-chunk
NPC = N // PC     # 8

F32 = mybir.dt.float32
F32R = mybir.dt.float32r

_CACHE = {}


def build_nc():
    nc = bacc.Bacc(None, target_bir_lowering=False, debug=False)

    # x^T arrives once in fp32, c-interleaved: xf[p, n, c] = x[n, 128c+p].
    xf = nc.declare_dram_parameter("xf", [128, N, DCH], F32, isOutput=False)
    # q/k weights hi/lo-split on host (cols 0:128 = hi both heads,
    # 128:256 = lo); v weight is hi-only (f32r-level accuracy suffices).
    wq = nc.declare_dram_parameter("wq", [D, 256], F32R, isOutput=False)
    wk = nc.declare_dram_parameter("wk", [D, 256], F32R, isOutput=False)
    wv = nc.declare_dram_parameter("wv", [D, 128], F32R, isOutput=False)
    wo = nc.declare_dram_parameter("wo", [128, D], F32R, isOutput=False)
    out = nc.declare_dram_parameter("out", [N, D], F32, isOutput=True)

    with ExitStack() as ctx:
        tc = ctx.enter_context(tile.TileContext(nc))
        singles = ctx.enter_context(tc.tile_pool(name="singles", bufs=1))
        bc_pool = ctx.enter_context(tc.tile_pool(name="bc", bufs=2))

        ident = singles.tile([128, 128], F32)
        make_identity(nc, ident)

        # long-lived SBUF tensors
        qT_ext = [singles.tile([65, N], F32R, tag=f"qT_ext{h}", name=f"qT_ext{h}")
                  for h in range(2)]
        kT_ext = [singles.tile([65, N], F32R, tag=f"kT_ext{h}", name=f"kT_ext{h}")
                  for h in range(2)]
        # stacked cross-term operands: one K=128 matmul computes
        # kl@qh + kh@ql.  qx = [qh; ql], kx = [kl; kh] (per head); the
        # hi rows double as the max-pass operands (no separate qTr/kTr).
        qx = [singles.tile([128, N], F32R, tag=f"qx{h}", name=f"qx{h}")
              for h in range(2)]
        kx = [singles.tile([128, N], F32R, tag=f"kx{h}", name=f"kx{h}")
              for h in range(2)]
        v_ext = [singles.tile([128, NMB, 65], F32R, tag=f"v_ext{h}",
                              name=f"v_ext{h}") for h in range(2)]
        mneg = [singles.tile([128, NQ], F32, tag=f"mneg{h}", name=f"mneg{h}")
                for h in range(2)]
        ctxn = singles.tile([128, N], F32R, tag="ctxn")
        wo_sb = singles.tile([128, D], F32R, tag="wo_sb")
        ones64r = singles.tile([1, 64], F32R, tag="ones64r")
        # per-chunk partial maxes: m4[qc][h][p, qbl, pair] (pair = 2 m-chunks)
        m4 = {}

        QHI, QLO = slice(0, 64), slice(64, 128)    # qx rows: [qh; ql]
        KLO, KHI = slice(0, 64), slice(64, 128)    # kx rows: [kl; kh]

        def mp_unit(pool, qc_t, qbl, mcs, comp, h):
            # one max-pass unit: len(mcs) hi-only S[q, m] matmuls
            # (512-wide m-chunks) into one psum tile, one XY-reduce ->
            # m4[qc_t][h][:, qbl, comp].  Hi operands come from the ext
            # tensors' rows 0:64 (partition-aligned; qx/kx store hi at
            # opposite halves).
            qb = qc_t * (QC // 128) + qbl
            t = pool.tile([128, 2, QC], F32, tag="mp", name="mp")
            for j, mc in enumerate(mcs):
                nc.tensor.matmul(
                    t[:, j, :],
                    qT_ext[h][0:64, qb * 128:(qb + 1) * 128],
                    kT_ext[h][0:64, mc * QC:(mc + 1) * QC],
                    start=True,
                    stop=True,
                )
            nc.vector.tensor_reduce(
                out=m4[qc_t][h][:, qbl, comp:comp + 1],
                in_=t[:, 0:len(mcs), :],
                axis=mybir.AxisListType.XY, op=mybir.AluOpType.max,
            )

        def mp_alloc(qc_t, ncomp):
            m4[qc_t] = [bc_pool.tile([128, NQ, ncomp], F32,
                                     tag=f"m4_{h}_{ncomp}", name=f"m4_{h}")
                        for h in range(2)]

        def mp_finish_reduce(qc):
            for h in range(2):
                # combine the component maxes per q-block, negated
                nc.vector.tensor_reduce(
                    out=mneg[h], in_=m4[qc][h],
                    axis=mybir.AxisListType.X, op=mybir.AluOpType.max,
                    negate=True,
                )

        def mp_finish_stage(qc, ptm_pool, ptm_tag, ptm_shape):
            qsl = slice(qc * QC, (qc + 1) * QC)
            # stage -max into qT_ext row 64: transpose [128, 4] -> [4, 128]
            # (rounded to f32r); the partition-major stream of [4, 128] is
            # exactly [1, 512].  The psum scratch borrows a slot of the
            # caller's pool via its standard tag (same slot bytes).
            for h in range(2):
                ptm = ptm_pool.tile(ptm_shape, F32, tag=ptm_tag, name="ptm",
                                    space="PSUM")
                nc.tensor.transpose(ptm[0:4, 0:128], mneg[h], ident)
                mt_sb = bc_pool.tile([4, 128], F32R, tag="mt_sb")
                nc.vector.tensor_copy(mt_sb, ptm[0:4, 0:128])
                nc.sync.dma_start(out=qT_ext[h][64:65, qsl], in_=mt_sb)

        # ---------------- phase 1: projections ----------------
        with tc.tile_pool(name="mp1", bufs=2, space="PSUM") as mp1, \
             tc.tile_pool(name="xs", bufs=2) as xs_pool, \
             tc.tile_pool(name="ph1", bufs=1) as ph1:
            ones_cols = ph1.tile([128, NMB, 1], F32)
            nc.vector.memset(ones_cols, 1.0)
            ones_row = ph1.tile([1, N], F32)
            nc.vector.memset(ones_row, 1.0)
            nc.gpsimd.tensor_copy(ones64r, ones_row[0:1, 0:64])

            w_sb = {
                "q": ph1.tile([128, DCH, 256], F32R, tag="w_q", name="w_q"),
                "k": ph1.tile([128, DCH, 256], F32R, tag="w_k", name="w_k"),
                "v": ph1.tile([128, DCH, 128], F32R, tag="w_v", name="w_v"),
            }
            wq_r = wq.rearrange("(c p) e -> p c e", p=128)
            wk_r = wk.rearrange("(c p) e -> p c e", p=128)
            wv_r = wv.rearrange("(c p) e -> p c e", p=128)

            with tc.tile_pool(name="pp", bufs=1, space="PSUM") as pp:
                for nchunk in range(NPC):
                    sl = slice(nchunk * PC, (nchunk + 1) * PC)
                    xft = xs_pool.tile([128, PC, DCH], F32, tag="xft", bufs=3)
                    xht = xs_pool.tile([128, PC, DCH], F32R, tag="xht")
                    xlt = xs_pool.tile([128, PC, DCH], F32R, tag="xlt")
                    # device-side hi/lo split: the f32r round-copy equals
                    # the PE's operand rounding; lo = x - hi (the PE
                    # re-rounds the lo operand on read, matching the
                    # host-side round11(x - hi)).  The sub splits 3/5
                    # across DVE/GpSimd so neither engine saturates.
                    nc.sync.dma_start(out=xft, in_=xf[:, sl, :])
                    if nchunk == 0:
                        # weights queue behind x chunk 0; wq per-c so
                        # matmul c starts as soon as its slice arrives
                        for c in range(DCH):
                            nc.sync.dma_start(out=w_sb["q"][:, c, :],
                                              in_=wq_r[:, c, :])
                        nc.sync.dma_start(out=w_sb["k"], in_=wk_r)
                        nc.sync.dma_start(out=w_sb["v"], in_=wv_r)
                        nc.sync.dma_start(out=wo_sb, in_=wo[:, :])
                    nc.vector.tensor_copy(xht, xft)
                    nc.vector.tensor_sub(xlt[:, :, 0:3], xft[:, :, 0:3],
                                         xht[:, :, 0:3])
                    nc.gpsimd.tensor_sub(xlt[:, :, 3:8], xft[:, :, 3:8],
                                         xht[:, :, 3:8])

                    # a chunk never starts with the kind the previous
                    # chunk ended with (psum bank still draining); chunk
                    # 7 keeps k before q (the staging units read kT)
                    names = ("q", "k", "v") if nchunk % 2 == 0 else \
                        ("k", "q", "v")
                    # max-pass pair(0,1) groups ride once kT[0:1024]
                    # lands (chunk 3); the pair(2,3) groups wait for
                    # chunk 7 / attention(0) where PE work covers them
                    ride = {
                        3: (0, (0, 1), 0, (0, 1)),
                        4: (0, (0, 1), 0, (2, 3)),
                        5: (1, (0, 1), 0, (0, 1)),
                        6: (1, (0, 1), 0, (2, 3)),
                    }.get(nchunk)
                    ride_units = []
                    if ride is not None:
                        qc_t, mcs, comp, qbls = ride
                        ride_units = [(qc_t, qbl, mcs, comp, h)
                                      for qbl in qbls for h in range(2)]
                    if nchunk == 3:
                        mp_alloc(0, 2)
                        mp_alloc(1, 2)
                    for name in names:
                        pt = pp.tile([128, PC], F32, tag=f"pt_{name}",
                                     name=f"pt_{name}",
                                     bufs=2 if name == "v" else 1)
                        terms = ((slice(0, 128), xht), (slice(0, 128), xlt),
                                 (slice(128, 256), xht))
                        if name == "v":
                            terms = ((slice(0, 128), xht),)
                        nmm = len(terms) * DCH
                        i = 0
                        for c in range(DCH):
                            for wsl, xt_ in terms:
                                nc.tensor.matmul(
                                    pt,
                                    w_sb[name][:, c, wsl],
                                    xt_[:, :, c],
                                    start=(i == 0),
                                    stop=(i == nmm - 1),
                                )
                                i += 1
                            if nchunk == NPC - 1 and name == "q":
                                # chunk-0's pair(2,3) staging units
                                # interleave with chunk 7's q matmuls so
                                # their DVE reduces never lock-step an
                                # idle PE
                                mp_unit(mp1, 0, c // 2, (2, 3), 1, c % 2)
                        if name == "v":
                            vT_c = xs_pool.tile([128, PC], F32, tag="vT_c")
                            nc.scalar.copy(out=vT_c, in_=pt)
                            # v_ext: transpose vT [64, 128-block] ->
                            # v [m, e] blocks [128, 64], inline per chunk
                            # (psum scratch reuses the pt_v slot bytes)
                            for nb2 in range(PC // 128):
                                mb = nchunk * (PC // 128) + nb2
                                for h in range(2):
                                    ptt = pp.tile([128, PC], F32,
                                                  tag="pt_v", name="ptt",
                                                  bufs=2)
                                    nc.tensor.transpose(
                                        ptt[:, 0:64],
                                        vT_c[h * 64:(h + 1) * 64,
                                             nb2 * 128:(nb2 + 1) * 128],
                                        ident[h * 64:(h + 1) * 64,
                                              h * 64:(h + 1) * 64],
                                    )
                                    nc.scalar.copy(out=v_ext[h][:, mb, 0:64],
                                                   in_=ptt[:, 0:64])
                            continue
                        dst_ext = qT_ext if name == "q" else kT_ext
                        dst_x = qx if name == "q" else kx
                        hi_rows = QHI if name == "q" else KHI
                        lo_rows = QLO if name == "q" else KLO
                        # unpack off DVE entirely (it owns the max-pass
                        # reduces): ACT copies psum to an fp32 scratch and
                        # to the rounded f32r hi; GpSimd (SBUF-only)
                        # derives the lo residual and the ext hi copy.
                        # The scratch lands at hi_rows so the sub's two
                        # inputs share a start partition (BIR verifier
                        # requirement for tensor_tensor).
                        for h in range(2):
                            hs = slice(h * 64, (h + 1) * 64)
                            tmp = xs_pool.tile([128, PC], F32,
                                               tag=f"tmp_{name}")
                            nc.scalar.copy(
                                out=tmp[hi_rows, :], in_=pt[hs, :])
                            nc.scalar.copy(
                                out=dst_x[h][hi_rows, sl], in_=pt[hs, :])
                            nc.gpsimd.tensor_sub(
                                dst_x[h][lo_rows, sl],
                                tmp[hi_rows, :], dst_x[h][hi_rows, sl])
                            nc.gpsimd.tensor_copy(
                                dst_ext[h][0:64, sl], dst_x[h][hi_rows, sl])
                    for u in ride_units:
                        mp_unit(mp1, *u)

            # post-proj: only chunk 0's staging chain (its unit reduces
            # are already queued); chunk 1's mc=3 group and finish move
            # into attention(0)'s tile schedule
            with tc.tile_pool(name="pv", bufs=1, space="PSUM") as pv:
                mp_finish_reduce(0)
                mp_finish_stage(0, pv, "ptm", [4, 128])
                for h in range(2):
                    # extension constants, written late (ACT is idle here
                    # and nothing reads them before attention): kT_ext
                    # row 64 = 1, v_ext col 64 = 1
                    nc.scalar.copy(out=kT_ext[h][64:65, :], in_=ones_row)
                    nc.scalar.copy(out=v_ext[h][:, :, 64:65], in_=ones_cols)

        # ---------------- phase 2: attention chunks ----------------
        # ex_pool holds attention-phase-only buffers (et/po_sb/po_f);
        # opening it AFTER the projection pools release keeps it out of
        # the projection-phase SBUF peak, paying for the triple-buffered
        # x stream
        ex_pool = ctx.enter_context(tc.tile_pool(name="ex", bufs=4))
        sp_ps = ctx.enter_context(tc.tile_pool(name="sp", bufs=3, space="PSUM"))
        ctx_pool = ctx.enter_context(tc.tile_pool(name="cx", bufs=1, space="PSUM"))
        mp2 = ctx.enter_context(tc.tile_pool(name="mp2", bufs=1, space="PSUM"))

        def norm_head(qc, h, ctx_ps, sliced=False):
            qsl = slice(qc * QC, (qc + 1) * QC)
            rz = bc_pool.tile([1, QC], F32, tag="rz")
            nc.vector.reciprocal(out=rz, in_=ctx_ps[h][64:65, :])
            bc_sb = bc_pool.tile([64, QC], F32, tag="bc_sb")
            nc.gpsimd.partition_broadcast(bc_sb, rz)
            hrows = slice(h * 64, (h + 1) * 64)
            if not sliced:
                nc.vector.tensor_mul(
                    ctxn[hrows, qsl], ctx_ps[h][0:64, :], bc_sb
                )
                return
            # final-chunk drain: 128-col slices so each o_proj block can
            # start as soon as its ctxn columns are normalized
            for s4 in range(4):
                ssl = slice(s4 * 128, (s4 + 1) * 128)
                nc.vector.tensor_mul(
                    ctxn[hrows, qc * QC + s4 * 128:qc * QC + (s4 + 1) * 128],
                    ctx_ps[h][0:64, ssl], bc_sb[:, ssl]
                )

        def norm_head_tail(qc, h, ctx_ps):
            # last-head normalize on the drain path: the partition
            # broadcast of 1/Z runs as a K=1 f32r PE matmul into the
            # idle max-pass psum slot (~0.2us vs ~1.3us on GpSimd), and
            # the muls are 128-col slices so each o_proj block starts as
            # soon as its columns land.  1/Z rounded to 11 bits costs
            # ~2e-4 relative — far inside the budget.
            rz = bc_pool.tile([1, QC], F32R, tag="rz_r")
            with nc.allow_low_precision(reason="1/Z at 11 bits, ~2e-4 rel"):
                nc.vector.reciprocal(out=rz, in_=ctx_ps[h][64:65, :])
            pot = mp2.tile([128, 2, QC], F32, tag="mp", name="bc_ps")
            bc_ps = pot[:, 0, :]
            nc.tensor.matmul(bc_ps[0:64, :], ones64r, rz,
                             start=True, stop=True)
            hrows = slice(h * 64, (h + 1) * 64)
            for s4 in range(4):
                ssl = slice(s4 * 128, (s4 + 1) * 128)
                nc.vector.tensor_mul(
                    ctxn[hrows, qc * QC + s4 * 128:qc * QC + (s4 + 1) * 128],
                    ctx_ps[h][0:64, ssl], bc_ps[0:64, ssl]
                )

        def oproj_block(qc, nb, fine_dma=False, on_act=False):
            # o_proj for one 128-row n-block (both heads fused: K=128);
            # the two 512-wide psum halves merge into one [128, 1024]
            # SBUF tile so the block is a single contiguous output DMA.
            # The psum->SBUF copy runs on ACT early on (DVE still has
            # max-pass work) and on DVE later (ACT is exp-saturated).
            n0 = qc * QC + nb * 128
            if fine_dma:
                # drain path: no score matmuls left, so the sp 3-bank
                # rotation pipelines the final o_proj matmuls; per-half
                # SBUF tiles (4-deep) and output DMAs alternating between
                # the two HWDGE queues (SP/ACT) keep the drain flowing
                for dc in range(D // QC):
                    po = sp_ps.tile([128, QC], F32, tag="sp", name="po")
                    nc.tensor.matmul(
                        po,
                        ctxn[:, n0:n0 + 128],
                        wo_sb[:, dc * QC:(dc + 1) * QC],
                        start=True,
                        stop=True,
                    )
                    pf = ex_pool.tile([128, QC], F32, tag="po_f", bufs=4)
                    if dc == 1:
                        nc.scalar.copy(out=pf, in_=po)
                    else:
                        nc.vector.tensor_copy(pf, po)
                    dma_eng = nc.sync if (2 * nb + dc) % 2 == 0 else nc.scalar
                    dma_eng.dma_start(
                        out=out[n0:n0 + 128, dc * QC:(dc + 1) * QC], in_=pf)
                return
            po_sb = ex_pool.tile([128, D], F32, tag="po_sb", bufs=2)
            for dc in range(D // QC):
                po = sp_ps.tile([128, QC], F32, tag="po", name="po", bufs=1)
                nc.tensor.matmul(
                    po,
                    ctxn[:, n0:n0 + 128],
                    wo_sb[:, dc * QC:(dc + 1) * QC],
                    start=True,
                    stop=True,
                )
                dst = po_sb[:, dc * QC:(dc + 1) * QC]
                if on_act:
                    nc.scalar.copy(out=dst, in_=po)
                else:
                    nc.vector.tensor_copy(dst, po)
            nc.sync.dma_start(out=out[n0:n0 + 128, :], in_=po_sb)

        def attention_chunk(qc, seq_heads=False):
            qsl = slice(qc * QC, (qc + 1) * QC)
            ctx_ps = [ctx_pool.tile([65, QC], F32, tag=f"ctx{h}",
                                    name=f"ctx_ps{h}") for h in range(2)]
            if not seq_heads:
                # h0's ctx matmuls lead so h1's norm (emitted last at the
                # previous chunk's end) has time to free its psum bank
                heads_order = ([(mb, 0) for mb in range(4)]
                               + [(mb, 1) for mb in range(4)]
                               + [(mb, h) for mb in range(4, NMB)
                                  for h in range(2)])
            else:
                heads_order = [(mb, h) for h in range(2) for mb in range(NMB)]
            started = {0: False, 1: False}

            # interleave schedules: max-pass units spread across chunks
            # so attention(0)'s entry (which inherits the post-proj DVE
            # backlog) carries only half of mp(2); mp(2)'s rest and
            # mp(3)'s first half share attention(1) with a LATE
            # finish(2), and mp(3) completes early in attention(2)
            def units_for(gen, qbls):
                return [(gen, qbl, (2 * pair, 2 * pair + 1), pair, h)
                        for qbl in qbls for pair in range(2)
                        for h in range(2)]

            mp_sched = {}
            fin_sched = {}
            if qc == 0:
                # chunk 1's mc=3 units lead (their reduces pace the DVE
                # queue while the crosses run), staging follows, then
                # half of mp(2)
                mp_alloc(2, 2)
                mc31 = [(1, qbl, (2, 3), 1, h) for qbl in range(4)
                        for h in range(2)]
                for t, u in zip((0, 2, 4, 6, 8, 10, 12, 14), mc31):
                    mp_sched[t] = u
                for t, u in zip((17, 18, 20, 22, 24, 26, 28, 30),
                                units_for(2, (0, 1))):
                    mp_sched[t] = u
                fin_sched[16] = lambda: mp_finish_reduce(1)
                fin_sched[21] = lambda: mp_finish_stage(
                    1, sp_ps, "sp", [128, QC])
            elif qc == 1:
                mp_alloc(3, 2)
                for t, u in zip((2, 4, 6, 8, 10, 12, 14, 16),
                                units_for(2, (2, 3))):
                    mp_sched[t] = u
                for t, u in zip((18, 20, 22, 24, 26, 28, 30, 31),
                                units_for(3, (0, 1))):
                    mp_sched[t] = u
                fin_sched[19] = lambda: mp_finish_reduce(2)
                fin_sched[25] = lambda: mp_finish_stage(
                    2, sp_ps, "sp", [128, QC])
            elif qc == 2:
                for t, u in zip((0, 2, 4, 6, 8, 10, 12, 14),
                                units_for(3, (2, 3))):
                    mp_sched[t] = u
                fin_sched[16] = lambda: mp_finish_reduce(3)
                fin_sched[21] = lambda: mp_finish_stage(
                    3, sp_ps, "sp", [128, QC])
            po_sched = {}
            if qc > 0:
                for nb in range(4):
                    po_sched[11 + 4 * nb] = (qc - 1, nb)

            def emit_m1_tail(sp, mb, h):
                # the only matmul that reads row 64 (the staged -max row);
                # lagging it one tile behind the cross matmul hides the
                # staging DMA latency at chunk entry
                nc.tensor.matmul(
                    sp, kT_ext[h][:, mb * 128:(mb + 1) * 128],
                    qT_ext[h][:, qsl],
                    start=False, stop=True,
                )
                et = ex_pool.tile([128, QC], F32R, tag="et", name="et")
                nc.scalar.activation(
                    out=et, in_=sp,
                    func=mybir.ActivationFunctionType.Exp, scale=0.125,
                )
                nc.tensor.matmul(
                    ctx_ps[h], v_ext[h][:, mb, :], et,
                    start=not started[h], stop=(mb == NMB - 1),
                )
                started[h] = True

            lagged = []
            for it, (mb, h) in enumerate(heads_order):
                if it in fin_sched:
                    fin_sched[it]()
                if it in mp_sched:
                    gen, *args = mp_sched[it]
                    mp_unit(mp2, gen, *args)
                if it in po_sched:
                    pqc, pnb = po_sched[it]
                    oproj_block(pqc, pnb, on_act=(qc == 1))
                msl = slice(mb * 128, (mb + 1) * 128)
                sp = sp_ps.tile([128, QC], F32, tag="sp", name=f"sp{h}")
                # stacked cross terms first (no row-64 dependency):
                # one K=128 matmul = kl@qh + kh@ql
                nc.tensor.matmul(
                    sp, kx[h][:, msl], qx[h][:, qsl],
                    start=True, stop=False,
                )
                lagged.append((sp, mb, h))
                if len(lagged) > 2:
                    emit_m1_tail(*lagged.pop(0))
                if seq_heads and mb == NMB - 1:
                    while lagged:
                        emit_m1_tail(*lagged.pop(0))
                    norm_head(qc, h, ctx_ps, sliced=(h == 1))
            while lagged:
                emit_m1_tail(*lagged.pop(0))
            return ctx_ps

        for qc in range(NQ):
            seq = qc == NQ - 1
            ctx_ps = attention_chunk(qc, seq_heads=seq)
            if not seq:
                for h in range(2):
                    norm_head(qc, h, ctx_ps)
        for nb in range(4):
            oproj_block(NQ - 1, nb, fine_dma=True)

    nc.compile()
    return nc


def _round11(x):
    # round-to-nearest-even to 11 explicit mantissa bits — exactly the
    # hardware's float32r operand rounding (verified on device)
    u = np.ascontiguousarray(x, dtype=np.float32).view(np.uint32)
    shift = 23 - 11
    add = np.uint32((1 << (shift - 1)) - 1)
    lsb = (u >> np.uint32(shift)) & np.uint32(1)
    mask = np.uint32(~((1 << shift) - 1) & 0xFFFFFFFF)
    return ((u + add + lsb) & mask).view(np.float32)


def _split11(x):
    hi = _round11(x)
    lo = _round11(x.astype(np.float32) - hi)
    return hi, lo


def kernel(x, q_proj, k_proj, v_proj, o_proj):
    if "nc" not in _CACHE:
        _CACHE["nc"] = build_nc()
    nc = _CACHE["nc"]

    # xf[p, n, c] = x[n, 128c+p]
    xf = np.ascontiguousarray(
        x.astype(np.float32, copy=False).reshape(N, DCH, 128).transpose(2, 0, 1)
    )
    in_maps = []
    for core in range(N_CORES):
        h0 = core * H_PER_CORE

        def wsplit(w):
            w2 = np.concatenate([w[h0], w[h0 + 1]], axis=1)  # [D, 128]
            wh, wl = _split11(w2)
            return np.ascontiguousarray(np.concatenate([wh, wl], axis=1))

        m = {
            "xf": xf,
            "wq": wsplit(q_proj),
            "wk": wsplit(k_proj),
            "wv": _round11(np.concatenate([v_proj[h0], v_proj[h0 + 1]],
                                          axis=1)),
            "wo": _round11(o_proj[h0 * 64:(h0 + 2) * 64, :]),
        }
        in_maps.append(m)

    try:
        res = run_bass_kernel_spmd(nc, in_maps, core_ids=list(range(N_CORES)))
    except Exception:
        # one retry: a fresh NRT session recovers transient device faults
        res = run_bass_kernel_spmd(nc, in_maps, core_ids=list(range(N_CORES)))
    _CACHE["last_results"] = res
    acc = np.zeros((N, D), dtype=np.float64)
    for core in range(N_CORES):
        acc += res.results[core]["out"].astype(np.float64)
    return acc.astype(np.float32)


if __name__ == "__main__":
    rng = np.random.default_rng(0)
    ins = {
        "x": rng.standard_normal((N, D), dtype=np.float32),
        "q_proj": rng.standard_normal((H, D, E), dtype=np.float32),
        "k_proj": rng.standard_normal((H, D, E), dtype=np.float32),
        "v_proj": rng.standard_normal((H, D, E), dtype=np.float32),
        "o_proj": rng.standard_normal((D, D), dtype=np.float32),
    }
    out = kernel(**ins)
    print("out", out.shape, out.dtype, np.abs(out).max())
